# revision 7
# baseline (speedup 1.0000x reference)
"""Trainium2 Bass kernel for a pre-LN transformer block (full-dim attention).

Sharding: 8 cores; core c handles batch b=c//2, sequence half h=c%2 (1024 query
rows). v5: each core computes k/v ONLY for its own 1024 rows; the pair of
cores covering one batch element exchanges k/v halves with a pairwise DRAM
AllGather (replica groups [[0,1],[2,3],[4,5],[6,7]]), eliminating the
duplicated k/v projection + LN1 work of earlier versions. The gathered kv is
in rank order (= natural sequence order), so causality is driven entirely by
host-supplied per-core exp-bias columns (keep/drop per kv chunk) and
triangular mask tiles (diagonal chunks).

Inherited from v2-v4: host-prepacked weights (all DMAs move 2KB-32KB
contiguous per-partition lines), residual x resident in SBUF (bf16),
software-pipelined attention (all score chunks issue before any y/proj so PE
never waits on softmax round-trips), single weight pass shared by both query
blocks in c_proj/MLP, v-bias folded into b_proj on the host (softmax rows
sum to 1).
"""

import sys
import time

import numpy as np

if "/opt/trn_rl_repo" not in sys.path:
    sys.path.insert(0, "/opt/trn_rl_repo")

P = 128
D = 1024
DC = D // P            # 8 feature chunks
T = 2048               # full kv sequence length
TOWN = 1024            # own (query) rows per core
TB = 512               # tile free-dim block
NKV = T // TB          # 4 kv blocks
NOWN = TOWN // TB      # 2 own blocks
FC = (4 * D) // P      # 32 fc chunks
EPS = 1e-5
ATT_SCALE = 0.125      # 1/sqrt(64)
NEG_BIAS = -60.0       # exp bias that zeroes dropped kv chunks

_CACHE = {}

# attention slot structure (SPMD-uniform): per q-block j2, the gathered kv
# chunks processed. Zigzag ownership: even cores own batch rows
# [0:512)+[1536:2048), odd cores own [512:1536). Gathered kv is rank-major:
# chunks 0-7 = even core's own rows (0-511 then 1536-2047), chunks 8-15 =
# odd core's rows (512-1535) -- identical chunk->row map on both cores.
# j2=0 (first own 512 rows): slots {0-3, 8-11}; j2=1: all 16 chunks.
# Mask-multiplied slots get host-driven triangular/ones tiles.
S_LIST = [[0, 1, 2, 3, 8, 9, 10, 11], list(range(16))]
MASK_SLOTS = [(0, 1, 2, 3, 8, 9, 10, 11), (4, 5, 6, 7, 12, 13, 14, 15)]
EB_BASE = [0, 8]


def _build_program():
    import concourse.bacc as bacc
    import concourse.mybir as mybir
    import concourse.tile as tile

    f32 = mybir.dt.float32
    f32r = mybir.dt.float32r
    bf16 = mybir.dt.bfloat16
    Alu = mybir.AluOpType
    Act = mybir.ActivationFunctionType

    nc = bacc.Bacc("TRN2", target_bir_lowering=False, debug=False)

    # ---- DRAM I/O (host-prepacked layouts; see _prepare_in_maps) ----
    xkv_pk = nc.dram_tensor("xkv_pk", [P, DC, TOWN], bf16,
                            kind="ExternalInput")
    wq_pk = nc.dram_tensor("wq_pk", [P, DC, DC, P], bf16, kind="ExternalInput")
    wk_pk = nc.dram_tensor("wk_pk", [P, DC, DC, P], bf16, kind="ExternalInput")
    wv_pk = nc.dram_tensor("wv_pk", [P, 2, DC, TB], bf16, kind="ExternalInput")
    wp_pk = nc.dram_tensor("wp_pk", [P, DC, DC, P], bf16, kind="ExternalInput")
    wfc_pk = nc.dram_tensor("wfc_pk", [P, FC, DC, P], bf16,
                            kind="ExternalInput")
    wfc2_pk = nc.dram_tensor("wfc2_pk", [P, 4, FC, 2, P], bf16,
                             kind="ExternalInput")
    g1pp = nc.dram_tensor("g1pp", [P, DC], f32, kind="ExternalInput")
    b1pp = nc.dram_tensor("b1pp", [P, DC], f32, kind="ExternalInput")
    g2pp = nc.dram_tensor("g2pp", [P, DC], f32, kind="ExternalInput")
    b2pp = nc.dram_tensor("b2pp", [P, DC], f32, kind="ExternalInput")
    bqk_pp = nc.dram_tensor("bqk_pp", [P, 2 * DC], f32, kind="ExternalInput")
    bproj_pp = nc.dram_tensor("bproj_pp", [P, DC], f32, kind="ExternalInput")
    bfc_pp = nc.dram_tensor("bfc_pp", [P, FC], f32, kind="ExternalInput")
    bfc2_pp = nc.dram_tensor("bfc2_pp", [P, DC], f32, kind="ExternalInput")
    ebias_pk = nc.dram_tensor("ebias_pk", [P, 24], f32, kind="ExternalInput")
    msk_pk = nc.dram_tensor("msk_pk", [8, P, TB], bf16, kind="ExternalInput")
    onesv = nc.dram_tensor("onesv", [P, 1], f32, kind="ExternalInput")
    onesv_bf = nc.dram_tensor("onesv_bf", [P, 1], bf16, kind="ExternalInput")
    out_pk = nc.dram_tensor("out_pk", [P, NOWN, 2, 4, TB], f32,
                            kind="ExternalOutput")

    # ---- kv exchange staging (pairwise AllGather, rank order) ----
    stg_k = nc.dram_tensor("stg_k", [DC, P, TOWN], bf16, kind="Internal")
    gat_k = nc.dram_tensor("gat_k", [2, DC, P, TOWN], bf16, kind="Internal")
    stg_v = nc.dram_tensor("stg_v", [DC, P, D], bf16, kind="Internal")
    gat_v = nc.dram_tensor("gat_v", [2, DC, P, D], bf16, kind="Internal")
    CC_GROUPS = [[0, 1], [2, 3], [4, 5], [6, 7]]

    with tile.TileContext(nc) as tc:
        import contextlib

        with contextlib.ExitStack() as ctx:
            persist = ctx.enter_context(tc.tile_pool(name="persist", bufs=1))
            psum = ctx.enter_context(tc.tile_pool(name="psum", bufs=1, space="PSUM"))

            # ---- critical-path loads first: x block 0 + the ones vector
            # feed the very first LN stats matmul ----
            xown = persist.tile([P, DC, TOWN], bf16, tag="xown", bufs=1,
                                name="xown")
            nc.sync.dma_start(xown[:, :, :TB], xkv_pk.ap()[:, :, :TB])
            ones_col_bf = persist.tile([P, 1], bf16, tag="ones_col_bf")
            nc.sync.dma_start(ones_col_bf[:], onesv_bf.ap())
            g1 = persist.tile([P, DC], f32, tag="g1")
            nc.sync.dma_start(g1[:], g1pp.ap())
            b1 = persist.tile([P, DC], f32, tag="b1")
            nc.sync.dma_start(b1[:], b1pp.ap())
            nc.sync.dma_start(xown[:, :, TB:2 * TB],
                              xkv_pk.ap()[:, :, TB:2 * TB])
            g2 = persist.tile([P, DC], f32, tag="g2")
            nc.sync.dma_start(g2[:], g2pp.ap())
            b2 = persist.tile([P, DC], f32, tag="b2")
            nc.sync.dma_start(b2[:], b2pp.ap())
            bqk = persist.tile([P, 2 * DC], f32, tag="bqk")
            nc.sync.dma_start(bqk[:], bqk_pp.ap())
            bproj = persist.tile([P, DC], f32, tag="bproj")
            nc.sync.dma_start(bproj[:], bproj_pp.ap())
            bfc = persist.tile([P, FC], f32, tag="bfc")
            nc.sync.dma_start(bfc[:], bfc_pp.ap())
            bfc2 = persist.tile([P, DC], f32, tag="bfc2")
            nc.sync.dma_start(bfc2[:], bfc2_pp.ap())
            ebias = persist.tile([P, 24], f32, tag="ebias")
            nc.sync.dma_start(ebias[:], ebias_pk.ap())
            ones_col = persist.tile([P, 1], f32r, tag="ones_col")
            nc.sync.dma_start(ones_col[:], onesv.ap().bitcast(f32r))
            ones_row_f = persist.tile([1, P], f32, tag="ones_row_f")
            nc.sync.dma_start(
                ones_row_f[:], onesv.ap().rearrange("(o p) c -> o (p c)", o=1)
            )
            epst = persist.tile([P, 1], f32, tag="epst")
            nc.vector.memset(epst[:], EPS)

            qT = {}   # (j2, dc) -> [P, TB] bf16 tile
            x2 = {}   # (j2, dd) -> [P, TB] bf16 tile
            h2 = {}   # (j2, dc) -> [P, TB] bf16 tile (LN2 output)

            def layernorm_t(pool, src_tiles, gt, bt, dst_dtype, tagp, h_pool=None, h_bufs=None, ones_t=None, sq_dt=None):
                """Transposed-layout layernorm over one TB block.

                src_tiles: callable dc -> [P, TB] AP (feature chunks on partitions).
                Returns list of 8 normalized [P, TB] tiles (dst_dtype).
                """
                if ones_t is None:
                    ones_t = ones_col
                if sq_dt is None:
                    sq_dt = f32r
                sum_ps = psum.tile([1, TB], f32, tag="small", bufs=2, name="sum_ps")
                sq_ps = psum.tile([1, TB], f32, tag="small", bufs=2, name="sq_ps")
                for dc in range(DC):
                    xsq = pool.tile([P, TB], sq_dt, tag=f"xsq{tagp}", bufs=2, name="xsq")
                    nc.scalar.activation(xsq[:], src_tiles(dc), Act.Square)
                    nc.tensor.matmul(
                        sum_ps[:], ones_t[:], src_tiles(dc),
                        start=(dc == 0), stop=(dc == DC - 1))
                    nc.tensor.matmul(
                        sq_ps[:], ones_t[:], xsq[:],
                        start=(dc == 0), stop=(dc == DC - 1))
                mu = pool.tile([1, TB], f32, tag=f"stat{tagp}", bufs=3, name="mu")
                nc.vector.tensor_scalar_mul(mu[:], sum_ps[:], 1.0 / D)
                var = pool.tile([1, TB], f32, tag=f"stat{tagp}", bufs=3, name="var")
                nc.vector.tensor_scalar_mul(var[:], sq_ps[:], 1.0 / D)
                mu2 = pool.tile([1, TB], f32, tag=f"stat{tagp}", bufs=3, name="mu2")
                nc.vector.tensor_mul(out=mu2[:], in0=mu[:], in1=mu[:])
                nc.vector.tensor_sub(out=var[:], in0=var[:], in1=mu2[:])
                std = pool.tile([1, TB], f32, tag=f"stat{tagp}", bufs=3, name="std")
                nc.scalar.activation(std[:], var[:], Act.Sqrt, bias=epst[:1])
                rstd = pool.tile([1, TB], f32, tag=f"stat{tagp}", bufs=3, name="rstd")
                nc.vector.reciprocal(rstd[:], std[:])
                mubc = psum.tile([P, TB], f32, tag="big", bufs=6, name="mubc")
                nc.tensor.matmul(mubc[:], ones_row_f[:], mu[:], start=True, stop=True)
                rsbc = psum.tile([P, TB], f32, tag="big", bufs=6, name="rsbc")
                nc.tensor.matmul(rsbc[:], ones_row_f[:], rstd[:], start=True, stop=True)
                outs = []
                for dc in range(DC):
                    u = pool.tile([P, TB], f32, tag=f"u{tagp}", bufs=1, name="u")
                    nc.vector.tensor_sub(out=u[:], in0=src_tiles(dc), in1=mubc[:])
                    nc.vector.tensor_mul(out=u[:], in0=u[:], in1=rsbc[:])
                    hp = h_pool if h_pool is not None else pool
                    h = hp.tile([P, TB], dst_dtype, tag=f"h{tagp}",
                                bufs=(h_bufs if h_bufs is not None else 2 * DC),
                                name="h")
                    nc.vector.tensor_scalar(
                        h[:], u[:], gt[:, dc:dc + 1], bt[:, dc:dc + 1],
                        Alu.mult, Alu.add)
                    outs.append(h)
                return outs

            # ================= Phases A+B =================
            phAB = tc.alloc_tile_pool(name="phAB", bufs=1)
            kT_res = [phAB.tile([P, T], bf16, tag="kTres", bufs=DC,
                                name=f"kTres{i}") for i in range(DC)]
            v_res = [phAB.tile([P, D], bf16, tag="vres", bufs=16,
                               name=f"vres{i}") for i in range(16)]
            msk = phAB.tile([P, 8, TB], bf16, tag="msk", bufs=1, name="msk")
            nc.sync.dma_start(msk[:], msk_pk.ap().rearrange("r p t -> p r t"))

            # ---- Phase A: LN1 + QKV (own rows) + pairwise kv exchange ----
            with tc.tile_pool(name="phA", bufs=1) as pa:
                h1 = {}

                def ln1_block(j):
                    h1[j] = layernorm_t(
                        pa,
                        lambda dc, jj=j: xown[:, dc, jj * TB:(jj + 1) * TB],
                        g1, b1, bf16, "A",
                        h_bufs=NOWN * DC, ones_t=ones_col_bf, sq_dt=bf16)

                for j in range(NOWN):
                    ln1_block(j)

                # k^T projection (own rows) -> stage -> AllGather
                kT_own = [pa.tile([P, TOWN], bf16, tag="kTown", bufs=DC,
                                  name=f"kTown{i}") for i in range(DC)]
                for nk in range(DC):
                    wk = pa.tile([P, DC, P], bf16, tag="wqk", bufs=2, name="wk")
                    nc.sync.dma_start(wk[:], wk_pk.ap()[:, nk])
                    for j in range(NOWN):
                        k_ps = psum.tile([P, TB], f32, tag="big", bufs=6,
                                         name="k_ps")
                        for dc in range(DC):
                            nc.tensor.matmul(
                                k_ps[:], wk[:, dc], h1[j][dc][:],
                                start=(dc == 0), stop=(dc == DC - 1))
                        nc.vector.tensor_scalar(
                            kT_own[nk][:, j * TB:(j + 1) * TB], k_ps[:],
                            bqk[:, DC + nk:DC + nk + 1], None, Alu.add)
                    nc.sync.dma_start(stg_k.ap()[nk], kT_own[nk][:])
                nc.gpsimd.collective_compute(
                    "AllGather", mybir.AluOpType.bypass,
                    replica_groups=CC_GROUPS,
                    ins=[stg_k.ap()], outs=[gat_k.ap()])

                # v projection (own rows) -> stage -> AllGather
                v_own = [pa.tile([P, D], bf16, tag="vown", bufs=DC,
                                 name=f"vown{i}") for i in range(DC)]
                for nvh in range(2):
                    wv = pa.tile([P, DC, TB], bf16, tag="wv", bufs=1,
                                 name="wv")
                    nc.sync.dma_start(wv[:], wv_pk.ap()[:, nvh])
                    for j in range(NOWN):
                        for sc in range(TB // P):
                            v_ps = psum.tile([P, TB], f32, tag="big", bufs=6,
                                             name="v_ps")
                            for dc in range(DC):
                                nc.tensor.matmul(
                                    v_ps[:],
                                    h1[j][dc][:, sc * P:(sc + 1) * P],
                                    wv[:, dc],
                                    start=(dc == 0), stop=(dc == DC - 1))
                            s_own = j * (TB // P) + sc
                            nc.vector.tensor_copy(
                                out=v_own[s_own][:, nvh * TB:(nvh + 1) * TB],
                                in_=v_ps[:])
                for s_own in range(DC):
                    nc.sync.dma_start(stg_v.ap()[s_own], v_own[s_own][:])
                nc.gpsimd.collective_compute(
                    "AllGather", mybir.AluOpType.bypass,
                    replica_groups=CC_GROUPS,
                    ins=[stg_v.ap()], outs=[gat_v.ap()])

                # q^T projection (own blocks) while the collectives fly
                for nq in range(DC):
                    wq = pa.tile([P, DC, P], bf16, tag="wqk", bufs=2,
                                 name="wq")
                    nc.sync.dma_start(wq[:], wq_pk.ap()[:, nq])
                    for j2 in range(NOWN):
                        q_ps = psum.tile([P, TB], f32, tag="big", bufs=6,
                                         name="q_ps")
                        for dc in range(DC):
                            nc.tensor.matmul(
                                q_ps[:], wq[:, dc], h1[j2][dc][:],
                                start=(dc == 0), stop=(dc == DC - 1))
                        qt = phAB.tile([P, TB], bf16, tag="qT",
                                       bufs=DC * NOWN, name="qt")
                        nc.vector.tensor_scalar(
                            qt[:], q_ps[:], bqk[:, nq:nq + 1], None,
                            Alu.add)
                        qT[(j2, nq)] = qt

                # gather results back into SBUF (rank order = natural order)
                for r in range(2):
                    for dc in range(DC):
                        nc.sync.dma_start(
                            kT_res[dc][:, r * TOWN:(r + 1) * TOWN],
                            gat_k.ap()[r, dc])
                for r in range(2):
                    for s in range(DC):
                        nc.sync.dma_start(v_res[r * DC + s][:],
                                          gat_v.ap()[r, s])

            # ---- Phase B: attention + c_proj + LN2 ----
            with tc.tile_pool(name="phB", bufs=1) as pb:
                att_tiles = {}
                rec = {}
                for j2 in range(NOWN):
                    mask_pos = {sp: i for i, sp in enumerate(MASK_SLOTS[j2])}
                    s_list = S_LIST[j2]
                    denom = psum.tile([1, TB], f32, tag="small", bufs=2,
                                      name="denom")
                    for idx, sp in enumerate(s_list):
                        sc_ps = psum.tile([P, TB], f32, tag="big", bufs=6,
                                          name="sc_ps")
                        for dc in range(DC):
                            nc.tensor.matmul(
                                sc_ps[:],
                                kT_res[dc][:, sp * P:(sp + 1) * P],
                                qT[(j2, dc)][:],
                                start=(dc == 0), stop=(dc == DC - 1))
                        att = pb.tile([P, TB], bf16, tag="att", bufs=24,
                                      name="att")
                        eb = EB_BASE[j2] + idx
                        nc.scalar.activation(
                            att[:], sc_ps[:], Act.Exp,
                            bias=ebias[:, eb:eb + 1], scale=ATT_SCALE)
                        if sp in mask_pos:
                            nc.vector.tensor_mul(
                                out=att[:], in0=att[:],
                                in1=msk[:, mask_pos[sp]])
                        nc.tensor.matmul(
                            denom[:], ones_col_bf[:], att[:],
                            start=(idx == 0), stop=(idx == len(s_list) - 1))
                        att_tiles[(j2, sp)] = att
                    rc = pb.tile([1, TB], f32, tag="rec", bufs=2, name="rc")
                    nc.vector.reciprocal(rc[:], denom[:])
                    rec[j2] = rc

                y_tiles = {}
                for j2 in range(NOWN):
                    rbc_ps = psum.tile([P, TB], f32, tag="big", bufs=6,
                                       name="rbc_ps")
                    nc.tensor.matmul(rbc_ps[:], ones_row_f[:], rec[j2][:],
                                     start=True, stop=True)
                    rbc = pb.tile([P, TB], f32, tag="rbc", bufs=2, name="rbc")
                    nc.vector.tensor_copy(out=rbc[:], in_=rbc_ps[:])
                    for dpass in range(2):
                        y_ps = [
                            psum.tile([P, TB], f32, tag="big", bufs=6,
                                      name="y_ps")
                            for _ in range(4)
                        ]
                        for idx, sp in enumerate(S_LIST[j2]):
                            for d4 in range(4):
                                dd = dpass * 4 + d4
                                nc.tensor.matmul(
                                    y_ps[d4][:],
                                    v_res[sp][:, dd * P:(dd + 1) * P],
                                    att_tiles[(j2, sp)][:],
                                    start=(idx == 0),
                                    stop=(idx == len(S_LIST[j2]) - 1))
                        for d4 in range(4):
                            yt = pb.tile([P, TB], bf16, tag="y", bufs=16,
                                         name="yt")
                            nc.vector.tensor_mul(
                                out=yt[:], in0=y_ps[d4][:], in1=rbc[:])
                            y_tiles[(j2, dpass * 4 + d4)] = yt

                # c_proj: one weight load serves both j2 halves
                for dd in range(DC):
                    wpt = pb.tile([P, DC, P], bf16, tag="wp", bufs=2,
                                  name="wpt")
                    nc.sync.dma_start(wpt[:], wp_pk.ap()[:, dd])
                    for j2 in range(NOWN):
                        p_ps = psum.tile([P, TB], f32, tag="big", bufs=6,
                                         name="p_ps")
                        for dc in range(DC):
                            nc.tensor.matmul(
                                p_ps[:], wpt[:, dc], y_tiles[(j2, dc)][:],
                                start=(dc == 0), stop=(dc == DC - 1))
                        x2t = persist.tile([P, TB], bf16, tag="x2",
                                           bufs=DC * NOWN, name="x2t")
                        nc.vector.scalar_tensor_tensor(
                            out=x2t[:], in0=p_ps[:],
                            scalar=bproj[:, dd:dd + 1],
                            in1=xown[:, dd, j2 * TB:(j2 + 1) * TB],
                            op0=Alu.add, op1=Alu.add)
                        x2[(j2, dd)] = x2t

                for j2 in range(NOWN):
                    h2j = layernorm_t(
                        pb, lambda dc, j=j2: x2[(j, dc)][:], g2, b2, bf16,
                        "C", h_pool=persist, h_bufs=DC * NOWN,
                        ones_t=ones_col_bf, sq_dt=bf16)
                    for dc in range(DC):
                        h2[(j2, dc)] = h2j[dc]

            phAB.release()

            # ================= Phase C: MLP =================
            with tc.tile_pool(name="phC", bufs=1) as pc:
                gel = {}
                for f in range(FC):
                    wf = pc.tile([P, DC, P], bf16, tag="wf", bufs=3,
                                 name="wf")
                    nc.sync.dma_start(wf[:], wfc_pk.ap()[:, f])
                    for j2 in range(NOWN):
                        fc_ps = psum.tile([P, TB], f32, tag="big", bufs=6,
                                          name="fc_ps")
                        for dc in range(DC):
                            nc.tensor.matmul(
                                fc_ps[:], wf[:, dc], h2[(j2, dc)][:],
                                start=(dc == 0), stop=(dc == DC - 1))
                        g = pc.tile([P, TB], bf16, tag="gel", bufs=2 * FC,
                                    name="g")
                        nc.scalar.activation(
                            g[:], fc_ps[:], Act.Gelu_apprx_tanh,
                            bias=bfc[:, f:f + 1])
                        gel[(j2, f)] = g
                for dpass in range(2):
                    for d4h in range(2):
                        q4 = dpass * 2 + d4h
                        y2_ps = [
                            psum.tile([P, TB], f32, tag="big", bufs=6,
                                      name="y2_ps")
                            for _ in range(4)
                        ]
                        for fq in range(4):
                            wf2 = pc.tile([P, 8, 2, P], bf16, tag="wf2",
                                          bufs=3, name="wf2")
                            nc.sync.dma_start(
                                wf2[:], wfc2_pk.ap()[:, q4, fq * 8:(fq + 1) * 8])
                            for fl in range(8):
                                f = fq * 8 + fl
                                for j2 in range(NOWN):
                                    for d4l in range(2):
                                        nc.tensor.matmul(
                                            y2_ps[j2 * 2 + d4l][:],
                                            wf2[:, fl, d4l], gel[(j2, f)][:],
                                            start=(f == 0), stop=(f == FC - 1))
                        for j2 in range(NOWN):
                            for d4l in range(2):
                                dd = dpass * 4 + d4h * 2 + d4l
                                ot = pc.tile([P, TB], f32, tag="outt", bufs=4,
                                             name="ot")
                                nc.vector.scalar_tensor_tensor(
                                    out=ot[:], in0=y2_ps[j2 * 2 + d4l][:],
                                    scalar=bfc2[:, dd:dd + 1],
                                    in1=x2[(j2, dd)][:],
                                    op0=Alu.add, op1=Alu.add)
                                nc.sync.dma_start(
                                    out_pk.ap()[:, j2, dpass,
                                                d4h * 2 + d4l], ot[:])

    nc.compile()
    return nc


def _prepare_in_maps(inputs):
    import ml_dtypes
    bf = ml_dtypes.bfloat16
    x = np.asarray(inputs["x"], dtype=np.float32)
    w_attn = np.ascontiguousarray(inputs["w_attn"], dtype=np.float32)
    w_proj = np.ascontiguousarray(inputs["w_proj"], dtype=np.float32)
    w_fc = np.ascontiguousarray(inputs["w_fc"], dtype=np.float32)
    w_fc2 = np.ascontiguousarray(inputs["w_fc2"], dtype=np.float32)
    b_attn = np.asarray(inputs["b_attn"], dtype=np.float32)
    b_proj = np.asarray(inputs["b_proj"], dtype=np.float32)
    b_fc = np.asarray(inputs["b_fc"], dtype=np.float32)
    b_fc2 = np.asarray(inputs["b_fc2"], dtype=np.float32)
    ln1_g = np.asarray(inputs["ln1_g"], dtype=np.float32)
    ln1_b = np.asarray(inputs["ln1_b"], dtype=np.float32)
    ln2_g = np.asarray(inputs["ln2_g"], dtype=np.float32)
    ln2_b = np.asarray(inputs["ln2_b"], dtype=np.float32)

    def pp(v, chunks):  # [chunks*P] -> [P, chunks] per-partition layout
        return np.ascontiguousarray(v.reshape(chunks, P).T)

    def pack_w(w):  # [D, dco*P] -> [P, dco, DC, P] tile-order pack
        dco = w.shape[1] // P
        return np.ascontiguousarray(
            w.reshape(DC, P, dco, P).transpose(1, 2, 0, 3)).astype(bf)

    mask4 = np.zeros((4, P, TB), np.float32)
    tri = np.triu(np.ones((P, P), np.float32))  # keep[s, t'] = t' >= s
    for r in range(4):
        for m in range(4):
            if r < m:
                mask4[r][:, m * P:(m + 1) * P] = 1.0
            elif r == m:
                mask4[r][:, m * P:(m + 1) * P] = tri

    shared = {
        "wq_pk": pack_w(w_attn[:, :D]),
        "wk_pk": pack_w(w_attn[:, D:2 * D]),
        "wv_pk": np.ascontiguousarray(
            w_attn[:, 2 * D:].reshape(DC, P, 2, TB).transpose(1, 2, 0, 3)
        ).astype(bf),
        "wp_pk": pack_w(w_proj),
        "wfc_pk": pack_w(w_fc),
        "wfc2_pk": np.ascontiguousarray(
            w_fc2.reshape(FC, P, 2, 2, 2, P).transpose(1, 2, 3, 0, 4, 5)
            .reshape(P, 4, FC, 2, P)
        ).astype(bf),
        "g1pp": pp(ln1_g, DC), "b1pp": pp(ln1_b, DC),
        "g2pp": pp(ln2_g, DC), "b2pp": pp(ln2_b, DC),
        "bqk_pp": pp(b_attn[:2 * D], 2 * DC),
        "bproj_pp": pp(b_proj + b_attn[2 * D:] @ w_proj, DC),
        "bfc_pp": pp(b_fc, FC),
        "bfc2_pp": pp(b_fc2, DC),
        "onesv": np.ones((P, 1), np.float32),
        "onesv_bf": np.ones((P, 1), bf),
    }

    ones_m = np.ones((P, TB), np.float32)

    # gathered kv chunk c covers rows: c<4 -> [128c, 128c+128) (even core's
    # low block); 4<=c<8 -> [1536+128(c-4), ...) (even core's high block);
    # c>=8 -> [512+128(c-8), ...) (odd core's middle rows).
    def chunk_rows(c):
        if c < 4:
            return c * P
        if c < 8:
            return 3 * TB + (c - 4) * P
        return TB + (c - 8) * P

    in_maps = []
    for c in range(8):
        b, h = c // 2, c % 2
        if h == 0:
            own_rows = np.r_[0:TB, 3 * TB:4 * TB]
        else:
            own_rows = np.r_[TB:3 * TB]
        own = x[b, own_rows]                         # [1024, D]
        xkv_pk = np.ascontiguousarray(
            own.T.reshape(DC, P, TOWN).transpose(1, 0, 2)).astype(bf)

        # per-slot exp bias: keep (0) / drop (NEG_BIAS). q-block j2 covers
        # own rows block j2 (zigzag): last absolute query row q_hi below.
        eb = np.zeros(24, np.float32)
        for j2 in range(NOWN):
            q_hi = own_rows[(j2 + 1) * TB - 1]
            for idx, sp in enumerate(S_LIST[j2]):
                if chunk_rows(sp) > q_hi:
                    eb[EB_BASE[j2] + idx] = NEG_BIAS
        ebias_arr = np.broadcast_to(eb, (P, 24)).copy()

        # mask tiles: diagonal chunks (own-block diagonals) triangular, rest
        # ones. msk[i] applies at slots MASK_SLOTS[j2][i]; the diagonal sits
        # at msk[0..3] for h=0 and msk[4..7] for h=1 for both j2.
        msk = np.ones((8, P, TB), np.float32)
        if h == 0:
            msk[0:4] = mask4
        else:
            msk[4:8] = mask4
        in_maps.append({**shared, "xkv_pk": xkv_pk,
                        "ebias_pk": ebias_arr,
                        "msk_pk": msk.astype(bf)})
    return in_maps


def _run(inputs, trace=False):
    from concourse import bass_utils

    if "nc" not in _CACHE:
        _CACHE["nc"] = _build_program()
    nc = _CACHE["nc"]
    in_maps = _prepare_in_maps(inputs)
    t0 = time.monotonic()
    res = bass_utils.run_bass_kernel_spmd(
        nc, in_maps, core_ids=list(range(8)), trace=trace)
    wall_ns = (time.monotonic() - t0) * 1e9

    x = np.asarray(inputs["x"])
    out = np.empty_like(x, dtype=np.float32)
    for c in range(8):
        b, h = c // 2, c % 2
        o = np.asarray(res.results[c]["out_pk"], dtype=np.float32)
        # [P, NOWN, 2, 4, TB] -> [D, TOWN]: row (dpass*4+d4)*P+p, col j2*TB+t
        full = o.reshape(P, NOWN, 2, 4, TB).transpose(2, 3, 0, 1, 4).reshape(
            D, TOWN)
        if h == 0:
            own_rows = np.r_[0:TB, 3 * TB:4 * TB]
        else:
            own_rows = np.r_[TB:3 * TB]
        out[b, own_rows, :] = full.T
    return out, res, wall_ns


def kernel(**inputs) -> np.ndarray:
    out, _, _ = _run(inputs, trace=False)
    return out
